# revision 19
# baseline (speedup 1.0000x reference)
"""DINO loss kernel for Trainium2 (8 NeuronCores, Bass/Tile).

Math: with S = student.reshape(640, D), T = teacher.reshape(128, D),
P = softmax((T - center)/tau), L = log_softmax(S/0.1), M = P @ L.T,
loss = -(sum(M) - trace(M)) / (128*639).

Decomposition (s = 10*S, c_v = logsumexp_d(s[v]), C = sum_v c_v):
  sum(M)   = dot(colsum_P, colsum_s) - 128*C
  trace(M) = sum_i (1/Z_i) sum_d E_i[d]*s_i[d]  -  C128
so each core computes, for its row blocks:
  - teacher E = exp(t - 40), row sums Z_i, colsum_P = sum_i E_i/Z_i
  - student logsumexp partials (exp(s - 30) row sums) and raw column sums
  - trace partials sum_d E[i,d]*S[i,d] for its 16 aligned student rows
Scalars/column-sums are combined on the host in f64.

Layout: inputs are cast to bf16 on the host (halves HBM traffic; rel-tol
is 2e-2 and the ~0.1-abs logit rounding washes out in the row-sum /
logsumexp averaging).  Rows are packed in 8-row blocks [128, 4096]:
partition p = r*16 + s, column c, d = s*4096 + c.  Column sums run on
the tensor engine with the tiny selector stationary (16 bf16 columns)
and the data moving 512 cols/matmul, writing 16-row bands of PSUM banks
at partition offsets {0,32,64}; the per-block partials accumulate in
PSUM, so each 65536-wide column sum ends in 3 banks -> 3 copies and 3
band DMAs.  Row sums of exp ride along free via activation accum_out.
The first teacher block is split in half so the first EXP starts as
early as possible, and a warmup activation preloads the Exp table
during the initial DMA.
"""

import numpy as np
import ml_dtypes

D = 65536
NCORES = 8
BLK = 4096               # free width of one 8-row block
W = BLK // 512           # 512-col matmul windows per block (8)
KT = 40.0                # teacher exp shift: exp(t - KT), cancels in softmax
KS = 30.0                # student exp shift: logsumexp = KS + log(sum exp(s-KS))

_CACHE = {}

TRACE = False            # test harness sets kernel.TRACE = True for profiling
LAST_RESULTS = None      # stashed BassKernelResults for the test harness


def _build_program():
    import concourse.bass as bass
    import concourse.tile as tile
    from concourse import bacc
    from concourse import mybir

    fp32 = mybir.dt.float32
    bf16 = mybir.dt.bfloat16
    nc = bacc.Bacc(None, target_bir_lowering=False)

    xt = nc.dram_tensor("xt", [128, 2 * BLK], bf16, kind="ExternalInput")
    xst = nc.dram_tensor("xst", [128, 2 * BLK], bf16, kind="ExternalInput")
    xsm = nc.dram_tensor("xsm", [128, 8 * BLK], bf16, kind="ExternalInput")
    b16 = nc.dram_tensor("b16", [128, 128], fp32, kind="ExternalInput")
    sel16 = nc.dram_tensor("sel16", [128, 16], bf16, kind="ExternalInput")

    # colsum outputs [j, s, 512q + c]: d = s*4096 + (3q+j)*512 + c, 3q+j < 8
    o_csp = nc.dram_tensor("csp", [3, 16, 1536], fp32, kind="ExternalOutput")
    o_cst = nc.dram_tensor("cst", [3, 16, 1536], fp32, kind="ExternalOutput")
    o_csm = nc.dram_tensor("csm", [3, 16, 1536], fp32, kind="ExternalOutput")
    # stats: cols 0-2 stz (block0 halves + block1), 3-4 stl, 5-12 stm,
    # 13-14 sttr
    o_stat = nc.dram_tensor("stat", [128, 17], fp32, kind="ExternalOutput")

    Exp = mybir.ActivationFunctionType.Exp
    MUL = mybir.AluOpType.mult
    ADD = mybir.AluOpType.add

    with tile.TileContext(nc) as tc:
        with (
            tc.tile_pool(name="singles", bufs=1) as singles,
            tc.tile_pool(name="loads", bufs=7) as loads,
            tc.tile_pool(name="scratch", bufs=3) as scratch,
            tc.tile_pool(name="stages", bufs=1) as stages,
            tc.tile_pool(name="psum", bufs=1, space="PSUM") as psum,
        ):
            bias_t = singles.tile([128, 1], fp32)
            nc.vector.memset(bias_t, -KT)
            bias_s = singles.tile([128, 1], fp32)
            nc.vector.memset(bias_s, -KS)
            # teacher tile DMAs issue first so the scalar engine starts ASAP;
            # block 0 in halves to cut the first EXP's wait.
            t0 = loads.tile([128, BLK], bf16, name="t0", tag="xload")
            Q4 = BLK // 4
            for h in range(4):
                nc.sync.dma_start(out=t0[:, h * Q4:(h + 1) * Q4],
                                  in_=xt[:, h * Q4:(h + 1) * Q4])
            t1 = loads.tile([128, BLK], bf16, name="t1", tag="xload")
            nc.sync.dma_start(out=t1, in_=xt[:, BLK:2 * BLK])
            # warm the Exp activation table while the first tile streams in
            warm = singles.tile([128, 1], fp32)
            nc.scalar.activation(out=warm, in_=bias_s, func=Exp,
                                 bias=bias_t, scale=1.0)

            e_res = singles.tile([128, 2 * BLK], bf16)   # teacher exp
            stat = singles.tile([128, 17], fp32)
            stz = stat[:, 0:5]
            stl = stat[:, 5:7]
            stm = stat[:, 7:15]
            sttr = stat[:, 15:17]

            # ---- teacher pass: E = exp(10*xt - 40), Z partials ----
            for h in range(4):
                nc.scalar.activation(
                    out=e_res[:, h * Q4:(h + 1) * Q4],
                    in_=t0[:, h * Q4:(h + 1) * Q4],
                    func=Exp, bias=bias_t, scale=10.0,
                    accum_out=stz[:, h:h + 1])
            nc.scalar.activation(
                out=e_res[:, BLK:2 * BLK], in_=t1,
                func=Exp, bias=bias_t, scale=10.0,
                accum_out=stz[:, 4:5])

            sel16t = singles.tile([128, 16], bf16)
            nc.sync.dma_start(out=sel16t, in_=sel16[:, :])
            b16t = singles.tile([128, 128], fp32)
            nc.sync.dma_start(out=b16t, in_=b16[:, :])

            # ---- rexp_b[p, m] = [p%16 == m] / Z_row(p) per block ----
            for h in range(3):
                nc.vector.tensor_add(stz[:, h + 1:h + 2], stz[:, h:h + 1],
                                     stz[:, h + 1:h + 2])
            zb_ps = psum.tile([128, 2], fp32, tag="pz")
            nc.tensor.matmul(zb_ps, b16t, stz[:, 3:5], start=True, stop=True)
            rb = singles.tile([128, 2], fp32)
            nc.vector.reciprocal(out=rb, in_=zb_ps)
            rexp = []
            for b in range(2):
                r = singles.tile([128, 16], bf16, name=f"rexp{b}")
                nc.vector.tensor_scalar_mul(
                    out=r, in0=sel16t, scalar1=rb[:, b:b + 1])
                rexp.append(r)

            # colsum matmul helper: window w=0..7 -> bank w//3, band 32*(w%3)
            def colsum(banks, stationary, moving_tile, b, nblk):
                for w in range(W):
                    q, off = w // 3, 32 * (w % 3)
                    nc.tensor.matmul(
                        banks[q][off:off + 16, :],
                        stationary[b] if isinstance(stationary, list)
                        else stationary,
                        moving_tile[:, 512 * w:512 * (w + 1)],
                        start=(b == 0), stop=(b == nblk - 1),
                    )

            deferred = []

            def flush(banks, out_dram, nm):
                st = stages.tile([128, 1536], fp32, name=f"st_{nm}",
                                 tag=f"stage_{nm}")
                for q in range(3):
                    nc.vector.tensor_copy(st[:, 512 * q:512 * (q + 1)],
                                          banks[q])
                for j in range(3):
                    deferred.append((out_dram[j, :, :],
                                     st[32 * j:32 * j + 16, :]))

            # ---- teacher colsum_P: sum_r E[r,:]/Z_r ----
            banks_p = [psum.tile([128, 512], fp32, name=f"bank_p{q}",
                                 tag=f"bA{q}") for q in range(3)]
            for b in range(2):
                colsum(banks_p, rexp, e_res[:, b * BLK:(b + 1) * BLK], b, 2)
            flush(banks_p, o_csp, "p")

            # ---- trace-student pass ----
            banks_t = [psum.tile([128, 512], fp32, name=f"bank_t{q}",
                                 tag=f"bB{q}") for q in range(3)]
            for b in range(2):
                xtile = loads.tile([128, BLK], bf16, tag="xload")
                nc.sync.dma_start(out=xtile, in_=xst[:, b * BLK:(b + 1) * BLK])
                sc = scratch.tile([128, BLK], bf16, tag="sc")
                nc.scalar.activation(
                    out=sc, in_=xtile, func=Exp, bias=bias_s, scale=10.0,
                    accum_out=stl[:, b:b + 1])
                sc2 = scratch.tile([128, BLK], bf16, tag="sc")
                nc.vector.scalar_tensor_tensor(
                    out=sc2, in0=e_res[:, b * BLK:(b + 1) * BLK],
                    scalar=1.0, in1=xtile, op0=MUL, op1=MUL,
                    accum_out=sttr[:, b:b + 1])
                colsum(banks_t, sel16t, xtile, b, 2)
            flush(banks_t, o_cst, "t")

            # ---- main-student pass ----
            banks_m = [psum.tile([128, 512], fp32, name=f"bank_m{q}",
                                 tag=f"bA{q}") for q in range(3)]
            for b in range(8):
                xtile = loads.tile([128, BLK], bf16, tag="xload")
                nc.sync.dma_start(out=xtile, in_=xsm[:, b * BLK:(b + 1) * BLK])
                sc = scratch.tile([128, BLK], bf16, tag="sc")
                nc.scalar.activation(
                    out=sc, in_=xtile, func=Exp, bias=bias_s, scale=10.0,
                    accum_out=stm[:, b:b + 1])
                colsum(banks_m, sel16t, xtile, b, 8)
            flush(banks_m, o_csm, "m")

            for out_ap, in_ap in deferred:
                nc.sync.dma_start(out=out_ap, in_=in_ap)
            nc.scalar.dma_start(out=o_stat[:, :], in_=stat)

    nc.compile()
    return nc


def _get_program():
    if "nc" not in _CACHE:
        _CACHE["nc"] = _build_program()
    return _CACHE["nc"]


def _selectors():
    sel16 = np.tile(np.eye(16, dtype=np.float32), (8, 1))
    b16 = np.kron(np.eye(8, dtype=np.float32), np.ones((16, 16), np.float32))
    return sel16.astype(ml_dtypes.bfloat16), b16


def _pack_blocks(rows_bf16):
    """[8k, 65536] bf16 -> [128, 4096k]: each 8-row block reshaped to
    [128, 4096] (p = r*16 + s), blocks concatenated along columns."""
    n = rows_bf16.shape[0] // 8
    blks = [np.ascontiguousarray(rows_bf16[8 * i:8 * (i + 1)].reshape(128, BLK))
            for i in range(n)]
    return np.concatenate(blks, axis=1) if n > 1 else blks[0]


def _unpack_colsum(cs):
    # cs [3, 16, 1536]: [j, s, 512q + c] -> d = s*4096 + (3q+j)*512 + c
    a = np.asarray(cs, dtype=np.float64).reshape(3, 16, 3, 512)
    vw = np.zeros((16, 8, 512))           # [s, w, c]
    for j in range(3):
        for q in range(3):
            w = 3 * q + j
            if w < W:
                vw[:, w, :] = a[j, :, q, :]
    return vw.reshape(D)


def _rowsum(stat_cols, nblk):
    # stat [128, nblk] accum partials, p = r*16 + s -> row = 8*b + r
    a = np.asarray(stat_cols, dtype=np.float64).reshape(8, 16, nblk)
    return a.sum(axis=1).T.reshape(8 * nblk)      # [b*8 + r]


def kernel(student_output, teacher_output, center, epoch):
    from concourse.bass_utils import run_bass_kernel_spmd

    global LAST_RESULTS

    S = np.asarray(student_output, dtype=np.float32).reshape(-1, D)   # [640, D]
    T = np.asarray(teacher_output, dtype=np.float32).reshape(-1, D)   # [128, D]
    cen = np.asarray(center, dtype=np.float32).reshape(1, D)
    ep = int(np.asarray(epoch))
    if ep < 30:
        t_temp = 0.04 + (0.07 - 0.04) * ep / 30
    else:
        t_temp = 0.07

    # host prep: fold center + temperature so the device uses one scale (10)
    tpre = ((T - cen) * np.float32(1.0 / (t_temp * 10.0))).astype(
        ml_dtypes.bfloat16)
    Sb = S.astype(ml_dtypes.bfloat16)

    sel16, b16 = _selectors()
    in_maps = []
    for k in range(NCORES):
        in_maps.append({
            "xt": _pack_blocks(tpre[16 * k:16 * (k + 1)]),
            "xst": _pack_blocks(Sb[16 * k:16 * (k + 1)]),
            "xsm": _pack_blocks(Sb[128 + 64 * k:128 + 64 * (k + 1)]),
            "b16": b16, "sel16": sel16,
        })

    nc = _get_program()
    res = run_bass_kernel_spmd(
        nc, in_maps, core_ids=list(range(NCORES)), trace=TRACE)
    LAST_RESULTS = res

    # host combine, all in float64
    colsum_P = np.zeros(D)
    colsum_sraw = np.zeros(D)
    C = 0.0
    C128 = 0.0
    TR = 0.0
    for k in range(NCORES):
        r = res.results[k]
        colsum_P += _unpack_colsum(r["csp"])
        colsum_sraw += _unpack_colsum(r["cst"])
        colsum_sraw += _unpack_colsum(r["csm"])

        stat = np.asarray(r["stat"], dtype=np.float64)
        z = _rowsum(stat[:, 3:5], 2)   # col3 = all block-0 quarters                    # teacher Z_i, 16 rows
        zs_tr = _rowsum(stat[:, 5:7], 2)      # trace-student exp sums
        zs_m = _rowsum(stat[:, 7:15], 8)      # main-student exp sums
        tr_acc = _rowsum(stat[:, 15:17], 2)   # sum_d E*S per trace row

        c_tr = KS + np.log(zs_tr)
        c_m = KS + np.log(zs_m)
        C += c_tr.sum() + c_m.sum()
        C128 += c_tr.sum()
        TR += (10.0 * tr_acc / z).sum()

    colsum_s = 10.0 * colsum_sraw
    s_pl = colsum_P @ colsum_s
    total = s_pl - 128.0 * C - TR + C128
    loss = -total / (128.0 * 639.0)
    return np.array(loss, dtype=np.float32)


# revision 20
# speedup vs baseline: 1.0029x; 1.0029x over previous
"""DINO loss kernel for Trainium2 (8 NeuronCores, Bass/Tile).

Math: with S = student.reshape(640, D), T = teacher.reshape(128, D),
P = softmax((T - center)/tau), L = log_softmax(S/0.1), M = P @ L.T,
loss = -(sum(M) - trace(M)) / (128*639).

Decomposition (s = 10*S, c_v = logsumexp_d(s[v]), C = sum_v c_v):
  sum(M)   = dot(colsum_P, colsum_s) - 128*C
  trace(M) = sum_i (1/Z_i) sum_d E_i[d]*s_i[d]  -  C128
so each core computes, for its row blocks:
  - teacher E = exp(t - 40), row sums Z_i, colsum_P = sum_i E_i/Z_i
  - student logsumexp partials (exp(s - 30) row sums) and raw column sums
  - trace partials sum_d E[i,d]*S[i,d] for its 16 aligned student rows
Scalars/column-sums are combined on the host in f64.

Layout: inputs are cast to bf16 on the host (halves HBM traffic; rel-tol
is 2e-2 and the ~0.1-abs logit rounding washes out in the row-sum /
logsumexp averaging).  Rows are packed in 8-row blocks [128, 4096]:
partition p = r*16 + s, column c, d = s*4096 + c.  Column sums run on
the tensor engine with the tiny selector stationary (16 bf16 columns)
and the data moving 512 cols/matmul, writing 16-row bands of PSUM banks
at partition offsets {0,32,64}; the per-block partials accumulate in
PSUM, so each 65536-wide column sum ends in 3 banks -> 3 copies and 3
band DMAs.  Row sums of exp ride along free via activation accum_out.
The first teacher block is split in half so the first EXP starts as
early as possible, and a warmup activation preloads the Exp table
during the initial DMA.
"""

import numpy as np
import ml_dtypes

D = 65536
NCORES = 8
BLK = 4096               # free width of one 8-row block
W = BLK // 512           # 512-col matmul windows per block (8)
KT = 40.0                # teacher exp shift: exp(t - KT), cancels in softmax
KS = 30.0                # student exp shift: logsumexp = KS + log(sum exp(s-KS))

_CACHE = {}

TRACE = False            # test harness sets kernel.TRACE = True for profiling
LAST_RESULTS = None      # stashed BassKernelResults for the test harness


def _build_program():
    import concourse.bass as bass
    import concourse.tile as tile
    from concourse import bacc
    from concourse import mybir

    fp32 = mybir.dt.float32
    bf16 = mybir.dt.bfloat16
    nc = bacc.Bacc(None, target_bir_lowering=False)

    xt = nc.dram_tensor("xt", [128, 2 * BLK], bf16, kind="ExternalInput")
    xst = nc.dram_tensor("xst", [128, 2 * BLK], bf16, kind="ExternalInput")
    xsm = nc.dram_tensor("xsm", [128, 8 * BLK], bf16, kind="ExternalInput")
    b16 = nc.dram_tensor("b16", [128, 128], fp32, kind="ExternalInput")
    sel16 = nc.dram_tensor("sel16", [128, 16], bf16, kind="ExternalInput")

    # colsum outputs [j, s, 512q + c]: d = s*4096 + (3q+j)*512 + c, 3q+j < 8
    o_csp = nc.dram_tensor("csp", [3, 16, 1536], fp32, kind="ExternalOutput")
    o_cst = nc.dram_tensor("cst", [3, 16, 1536], fp32, kind="ExternalOutput")
    o_csm = nc.dram_tensor("csm", [3, 16, 1536], fp32, kind="ExternalOutput")
    # stats: cols 0-2 stz (block0 halves + block1), 3-4 stl, 5-12 stm,
    # 13-14 sttr
    o_stat = nc.dram_tensor("stat", [128, 15], fp32, kind="ExternalOutput")

    Exp = mybir.ActivationFunctionType.Exp
    MUL = mybir.AluOpType.mult
    ADD = mybir.AluOpType.add

    with tile.TileContext(nc) as tc:
        with (
            tc.tile_pool(name="singles", bufs=1) as singles,
            tc.tile_pool(name="loads", bufs=7) as loads,
            tc.tile_pool(name="scratch", bufs=3) as scratch,
            tc.tile_pool(name="stages", bufs=1) as stages,
            tc.tile_pool(name="psum", bufs=1, space="PSUM") as psum,
        ):
            bias_t = singles.tile([128, 1], fp32)
            nc.vector.memset(bias_t, -KT)
            bias_s = singles.tile([128, 1], fp32)
            nc.vector.memset(bias_s, -KS)
            # teacher tile DMAs issue first so the scalar engine starts ASAP;
            # block 0 in halves to cut the first EXP's wait.
            t0 = loads.tile([128, BLK], bf16, name="t0", tag="xload")
            H2 = BLK // 2
            for h in range(2):
                nc.sync.dma_start(out=t0[:, h * H2:(h + 1) * H2],
                                  in_=xt[:, h * H2:(h + 1) * H2])
            t1 = loads.tile([128, BLK], bf16, name="t1", tag="xload")
            nc.sync.dma_start(out=t1, in_=xt[:, BLK:2 * BLK])
            # warm the Exp activation table while the first tile streams in
            warm = singles.tile([128, 1], fp32)
            nc.scalar.activation(out=warm, in_=bias_s, func=Exp,
                                 bias=bias_t, scale=1.0)

            e_res = singles.tile([128, 2 * BLK], bf16)   # teacher exp
            stat = singles.tile([128, 15], fp32)
            stz = stat[:, 0:3]
            stl = stat[:, 3:5]
            stm = stat[:, 5:13]
            sttr = stat[:, 13:15]

            # ---- teacher pass: E = exp(10*xt - 40), Z partials ----
            for h in range(2):
                nc.scalar.activation(
                    out=e_res[:, h * H2:(h + 1) * H2],
                    in_=t0[:, h * H2:(h + 1) * H2],
                    func=Exp, bias=bias_t, scale=10.0,
                    accum_out=stz[:, h:h + 1])
            nc.scalar.activation(
                out=e_res[:, BLK:2 * BLK], in_=t1,
                func=Exp, bias=bias_t, scale=10.0,
                accum_out=stz[:, 2:3])

            sel16t = singles.tile([128, 16], bf16)
            nc.sync.dma_start(out=sel16t, in_=sel16[:, :])
            b16t = singles.tile([128, 128], fp32)
            nc.sync.dma_start(out=b16t, in_=b16[:, :])

            # ---- rexp_b[p, m] = [p%16 == m] / Z_row(p) per block ----
            nc.vector.tensor_add(stz[:, 1:2], stz[:, 0:1], stz[:, 1:2])
            zb_ps = psum.tile([128, 2], fp32, tag="pz")
            nc.tensor.matmul(zb_ps, b16t, stz[:, 1:3], start=True, stop=True)
            rb = singles.tile([128, 2], fp32)
            nc.vector.reciprocal(out=rb, in_=zb_ps)
            rexp = []
            for b in range(2):
                r = singles.tile([128, 16], bf16, name=f"rexp{b}")
                nc.vector.tensor_scalar_mul(
                    out=r, in0=sel16t, scalar1=rb[:, b:b + 1])
                rexp.append(r)

            # colsum matmul helper: window w=0..7 -> bank w//3, band 32*(w%3)
            def colsum(banks, stationary, moving_tile, b, nblk):
                for w in range(W):
                    q, off = w // 3, 32 * (w % 3)
                    nc.tensor.matmul(
                        banks[q][off:off + 16, :],
                        stationary[b] if isinstance(stationary, list)
                        else stationary,
                        moving_tile[:, 512 * w:512 * (w + 1)],
                        start=(b == 0), stop=(b == nblk - 1),
                    )

            deferred = []

            def flush(banks, out_dram, nm):
                st = stages.tile([128, 1536], fp32, name=f"st_{nm}",
                                 tag=f"stage_{nm}")
                for q in range(3):
                    nc.vector.tensor_copy(st[:, 512 * q:512 * (q + 1)],
                                          banks[q])
                for j in range(3):
                    deferred.append((out_dram[j, :, :],
                                     st[32 * j:32 * j + 16, :]))

            # ---- teacher colsum_P: sum_r E[r,:]/Z_r ----
            banks_p = [psum.tile([128, 512], fp32, name=f"bank_p{q}",
                                 tag=f"bA{q}") for q in range(3)]
            for b in range(2):
                colsum(banks_p, rexp, e_res[:, b * BLK:(b + 1) * BLK], b, 2)
            flush(banks_p, o_csp, "p")

            # ---- trace-student pass ----
            banks_t = [psum.tile([128, 512], fp32, name=f"bank_t{q}",
                                 tag=f"bB{q}") for q in range(3)]
            for b in range(2):
                xtile = loads.tile([128, BLK], bf16, tag="xload")
                nc.sync.dma_start(out=xtile, in_=xst[:, b * BLK:(b + 1) * BLK])
                sc = scratch.tile([128, BLK], bf16, tag="sc")
                nc.scalar.activation(
                    out=sc, in_=xtile, func=Exp, bias=bias_s, scale=10.0,
                    accum_out=stl[:, b:b + 1])
                sc2 = scratch.tile([128, BLK], bf16, tag="sc")
                nc.vector.scalar_tensor_tensor(
                    out=sc2, in0=e_res[:, b * BLK:(b + 1) * BLK],
                    scalar=1.0, in1=xtile, op0=MUL, op1=MUL,
                    accum_out=sttr[:, b:b + 1])
                colsum(banks_t, sel16t, xtile, b, 2)
            flush(banks_t, o_cst, "t")

            # ---- main-student pass ----
            banks_m = [psum.tile([128, 512], fp32, name=f"bank_m{q}",
                                 tag=f"bA{q}") for q in range(3)]
            for b in range(8):
                xtile = loads.tile([128, BLK], bf16, tag="xload")
                nc.sync.dma_start(out=xtile, in_=xsm[:, b * BLK:(b + 1) * BLK])
                sc = scratch.tile([128, BLK], bf16, tag="sc")
                nc.scalar.activation(
                    out=sc, in_=xtile, func=Exp, bias=bias_s, scale=10.0,
                    accum_out=stm[:, b:b + 1])
                colsum(banks_m, sel16t, xtile, b, 8)
            flush(banks_m, o_csm, "m")

            for out_ap, in_ap in deferred:
                nc.sync.dma_start(out=out_ap, in_=in_ap)
            nc.scalar.dma_start(out=o_stat[:, :], in_=stat)

    nc.compile()
    return nc


def _get_program():
    if "nc" not in _CACHE:
        _CACHE["nc"] = _build_program()
    return _CACHE["nc"]


def _selectors():
    sel16 = np.tile(np.eye(16, dtype=np.float32), (8, 1))
    b16 = np.kron(np.eye(8, dtype=np.float32), np.ones((16, 16), np.float32))
    return sel16.astype(ml_dtypes.bfloat16), b16


def _pack_blocks(rows_bf16):
    """[8k, 65536] bf16 -> [128, 4096k]: each 8-row block reshaped to
    [128, 4096] (p = r*16 + s), blocks concatenated along columns."""
    n = rows_bf16.shape[0] // 8
    blks = [np.ascontiguousarray(rows_bf16[8 * i:8 * (i + 1)].reshape(128, BLK))
            for i in range(n)]
    return np.concatenate(blks, axis=1) if n > 1 else blks[0]


def _unpack_colsum(cs):
    # cs [3, 16, 1536]: [j, s, 512q + c] -> d = s*4096 + (3q+j)*512 + c
    a = np.asarray(cs, dtype=np.float64).reshape(3, 16, 3, 512)
    vw = np.zeros((16, 8, 512))           # [s, w, c]
    for j in range(3):
        for q in range(3):
            w = 3 * q + j
            if w < W:
                vw[:, w, :] = a[j, :, q, :]
    return vw.reshape(D)


def _rowsum(stat_cols, nblk):
    # stat [128, nblk] accum partials, p = r*16 + s -> row = 8*b + r
    a = np.asarray(stat_cols, dtype=np.float64).reshape(8, 16, nblk)
    return a.sum(axis=1).T.reshape(8 * nblk)      # [b*8 + r]


def kernel(student_output, teacher_output, center, epoch):
    from concourse.bass_utils import run_bass_kernel_spmd

    global LAST_RESULTS

    S = np.asarray(student_output, dtype=np.float32).reshape(-1, D)   # [640, D]
    T = np.asarray(teacher_output, dtype=np.float32).reshape(-1, D)   # [128, D]
    cen = np.asarray(center, dtype=np.float32).reshape(1, D)
    ep = int(np.asarray(epoch))
    if ep < 30:
        t_temp = 0.04 + (0.07 - 0.04) * ep / 30
    else:
        t_temp = 0.07

    # host prep: fold center + temperature so the device uses one scale (10)
    tpre = ((T - cen) * np.float32(1.0 / (t_temp * 10.0))).astype(
        ml_dtypes.bfloat16)
    Sb = S.astype(ml_dtypes.bfloat16)

    sel16, b16 = _selectors()
    in_maps = []
    for k in range(NCORES):
        in_maps.append({
            "xt": _pack_blocks(tpre[16 * k:16 * (k + 1)]),
            "xst": _pack_blocks(Sb[16 * k:16 * (k + 1)]),
            "xsm": _pack_blocks(Sb[128 + 64 * k:128 + 64 * (k + 1)]),
            "b16": b16, "sel16": sel16,
        })

    nc = _get_program()
    res = run_bass_kernel_spmd(
        nc, in_maps, core_ids=list(range(NCORES)), trace=TRACE)
    LAST_RESULTS = res

    # host combine, all in float64
    colsum_P = np.zeros(D)
    colsum_sraw = np.zeros(D)
    C = 0.0
    C128 = 0.0
    TR = 0.0
    for k in range(NCORES):
        r = res.results[k]
        colsum_P += _unpack_colsum(r["csp"])
        colsum_sraw += _unpack_colsum(r["cst"])
        colsum_sraw += _unpack_colsum(r["csm"])

        stat = np.asarray(r["stat"], dtype=np.float64)
        z = _rowsum(stat[:, 1:3], 2)   # col1 = both block-0 halves                    # teacher Z_i, 16 rows
        zs_tr = _rowsum(stat[:, 3:5], 2)      # trace-student exp sums
        zs_m = _rowsum(stat[:, 5:13], 8)      # main-student exp sums
        tr_acc = _rowsum(stat[:, 13:15], 2)   # sum_d E*S per trace row

        c_tr = KS + np.log(zs_tr)
        c_m = KS + np.log(zs_m)
        C += c_tr.sum() + c_m.sum()
        C128 += c_tr.sum()
        TR += (10.0 * tr_acc / z).sum()

    colsum_s = 10.0 * colsum_sraw
    s_pl = colsum_P @ colsum_s
    total = s_pl - 128.0 * C - TR + C128
    loss = -total / (128.0 * 639.0)
    return np.array(loss, dtype=np.float32)


# revision 22
# speedup vs baseline: 1.1880x; 1.1846x over previous
"""DINO loss kernel for Trainium2 (8 NeuronCores, Bass/Tile).

Math: with S = student.reshape(640, D), T = teacher.reshape(128, D),
P = softmax((T - center)/tau), L = log_softmax(S/0.1), M = P @ L.T,
loss = -(sum(M) - trace(M)) / (128*639).

Decomposition (s = 10*S, c_v = logsumexp_d(s[v]), C = sum_v c_v):
  sum(M)   = dot(colsum_P, colsum_s) - 128*C
  trace(M) = sum_i (1/Z_i) sum_d E_i[d]*s_i[d]  -  C128
so each core computes, for its row blocks:
  - teacher E = exp(t - 40), row sums Z_i, colsum_P = sum_i E_i/Z_i
  - student logsumexp partials (exp(s - 30) row sums) and raw column sums
  - trace partials sum_d E[i,d]*S[i,d] for its 16 aligned student rows
Scalars/column-sums are combined on the host in f64.

Layout: inputs are cast to bf16 on the host (halves HBM traffic; rel-tol
is 2e-2 and the ~0.1-abs logit rounding washes out in the row-sum /
logsumexp averaging).  Rows are packed in 16-row blocks [128, 8192]:
partition p = r*8 + s, column c, d = s*8192 + c.  Column sums run on the
tensor engine with the tiny selector stationary (8 bf16 columns) and the
data moving 512 cols/matmul, writing 8-row bands of PSUM banks at
partition offsets {0,32,64}; main-student block partials accumulate in
PSUM.  Each 65536-wide column sum lands in 6 banks -> 6 copies and 3
band DMAs.  Row sums of exp ride along free via activation accum_out;
col-range splits of one activation accumulate to separate stat columns
that are summed afterwards (rows stay per-partition under col splits).
The teacher block is split so the first EXP starts as early as possible,
and a warmup activation preloads the Exp table during the initial DMA.
"""

import numpy as np
import ml_dtypes

D = 65536
NCORES = 8
BLK = 8192               # free width of one 16-row block
W = BLK // 512           # 512-col matmul windows per block (16)
NQ = 6                   # psum banks per colsum set (ceil(16/3))
KT = 40.0                # teacher exp shift: exp(t - KT), cancels in softmax
KS = 30.0                # student exp shift: logsumexp = KS + log(sum exp(s-KS))

_CACHE = {}

TRACE = False            # test harness sets kernel.TRACE = True for profiling
LAST_RESULTS = None      # stashed BassKernelResults for the test harness


def _build_program():
    import concourse.bass as bass
    import concourse.tile as tile
    from concourse import bacc
    from concourse import mybir

    fp32 = mybir.dt.float32
    bf16 = mybir.dt.bfloat16
    nc = bacc.Bacc(None, target_bir_lowering=False)

    xt = nc.dram_tensor("xt", [128, BLK], bf16, kind="ExternalInput")
    xst = nc.dram_tensor("xst", [128, BLK], bf16, kind="ExternalInput")
    xsm = nc.dram_tensor("xsm", [128, 4 * BLK], bf16, kind="ExternalInput")
    b8 = nc.dram_tensor("b8", [128, 128], fp32, kind="ExternalInput")
    sel8 = nc.dram_tensor("sel8", [128, 8], bf16, kind="ExternalInput")

    # colsum outputs [j, s, 512q + c]: d = s*8192 + (3q+j)*512 + c, 3q+j < 16
    o_csp = nc.dram_tensor("csp", [3, 8, 512 * NQ], fp32,
                           kind="ExternalOutput")
    o_cst = nc.dram_tensor("cst", [3, 8, 512 * NQ], fp32,
                           kind="ExternalOutput")
    o_csm = nc.dram_tensor("csm", [3, 8, 512 * NQ], fp32,
                           kind="ExternalOutput")
    # stats: cols 0-1 stz halves, 2-3 stl halves, 4-7 stm, 8-9 sttr halves
    o_stat = nc.dram_tensor("stat", [128, 10], fp32, kind="ExternalOutput")

    Exp = mybir.ActivationFunctionType.Exp
    MUL = mybir.AluOpType.mult

    with tile.TileContext(nc) as tc:
        with (
            tc.tile_pool(name="singles", bufs=1) as singles,
            tc.tile_pool(name="loads", bufs=4) as loads,
            tc.tile_pool(name="scratch", bufs=3) as scratch,
            tc.tile_pool(name="stages", bufs=1) as stages,
            tc.tile_pool(name="psum", bufs=1, space="PSUM") as psum,
        ):
            bias_t = singles.tile([128, 1], fp32)
            nc.vector.memset(bias_t, -KT)
            bias_s = singles.tile([128, 1], fp32)
            nc.vector.memset(bias_s, -KS)
            # teacher tile DMA issues first, in col-quarters, so the scalar
            # engine starts ASAP (its EXP runs as two col-halves).
            t0 = loads.tile([128, BLK], bf16, name="t0", tag="xload")
            Q4 = BLK // 4
            for h in range(4):
                nc.sync.dma_start(out=t0[:, h * Q4:(h + 1) * Q4],
                                  in_=xt[:, h * Q4:(h + 1) * Q4])
            # warm the Exp activation table while the first tile streams in
            warm = singles.tile([128, 1], fp32)
            nc.scalar.activation(out=warm, in_=bias_s, func=Exp,
                                 bias=bias_t, scale=1.0)

            e_res = singles.tile([128, BLK], bf16)   # teacher exp
            stat = singles.tile([128, 10], fp32)
            stz = stat[:, 0:2]
            stl = stat[:, 2:4]
            stm = stat[:, 4:8]
            sttr = stat[:, 8:10]

            # ---- teacher pass: E = exp(10*xt - 40), Z half-partials ----
            H2 = BLK // 2
            for h in range(2):
                nc.scalar.activation(
                    out=e_res[:, h * H2:(h + 1) * H2],
                    in_=t0[:, h * H2:(h + 1) * H2],
                    func=Exp, bias=bias_t, scale=10.0,
                    accum_out=stz[:, h:h + 1])

            sel8t = singles.tile([128, 8], bf16)
            nc.sync.dma_start(out=sel8t, in_=sel8[:, :])
            b8t = singles.tile([128, 128], fp32)
            nc.sync.dma_start(out=b8t, in_=b8[:, :])

            # ---- rexp[p, m] = [p%8 == m] / Z_row(p) ----
            nc.vector.tensor_add(stz[:, 1:2], stz[:, 0:1], stz[:, 1:2])
            zb_ps = psum.tile([128, 1], fp32, tag="pz")
            nc.tensor.matmul(zb_ps, b8t, stz[:, 1:2], start=True, stop=True)
            rb = singles.tile([128, 1], fp32)
            nc.vector.reciprocal(out=rb, in_=zb_ps)
            rexp = singles.tile([128, 8], bf16)
            nc.vector.tensor_scalar_mul(out=rexp, in0=sel8t, scalar1=rb)

            # colsum matmuls: window w=0..15 -> bank w//3, band 32*(w%3)
            def colsum(banks, stationary, moving_tile, b, nblk):
                for w in range(W):
                    q, off = w // 3, 32 * (w % 3)
                    nc.tensor.matmul(
                        banks[q][off:off + 8, :],
                        stationary,
                        moving_tile[:, 512 * w:512 * (w + 1)],
                        start=(b == 0), stop=(b == nblk - 1),
                    )

            deferred = []

            def flush(banks, out_dram, nm):
                st = stages.tile([128, 512 * NQ], fp32, name=f"st_{nm}",
                                 tag=f"stage_{nm}")
                for q in range(NQ):
                    nc.vector.tensor_copy(st[:, 512 * q:512 * (q + 1)],
                                          banks[q])
                for j in range(3):
                    deferred.append((out_dram[j, :, :],
                                     st[32 * j:32 * j + 8, :]))

            def mkbanks(nm):
                return [psum.tile([128, 512], fp32, name=f"bank_{nm}{q}",
                                  tag=f"bk{q}") for q in range(NQ)]

            # ---- teacher colsum_P: sum_r E[r,:]/Z_r ----
            banks_p = mkbanks("p")
            colsum(banks_p, rexp, e_res, 0, 1)
            flush(banks_p, o_csp, "p")

            # ---- trace-student pass ----
            xst_t = loads.tile([128, BLK], bf16, name="xst_t", tag="xload")
            nc.sync.dma_start(out=xst_t, in_=xst[:, :])
            sc_t = scratch.tile([128, BLK], bf16, tag="sc")
            for h in range(2):
                nc.scalar.activation(
                    out=sc_t[:, h * H2:(h + 1) * H2],
                    in_=xst_t[:, h * H2:(h + 1) * H2],
                    func=Exp, bias=bias_s, scale=10.0,
                    accum_out=stl[:, h:h + 1])
            sc2 = scratch.tile([128, BLK], bf16, tag="sc")
            for h in range(2):
                nc.vector.scalar_tensor_tensor(
                    out=sc2[:, h * H2:(h + 1) * H2],
                    in0=e_res[:, h * H2:(h + 1) * H2],
                    scalar=1.0, in1=xst_t[:, h * H2:(h + 1) * H2],
                    op0=MUL, op1=MUL,
                    accum_out=sttr[:, h:h + 1])
            banks_t = mkbanks("t")
            colsum(banks_t, sel8t, xst_t, 0, 1)
            flush(banks_t, o_cst, "t")

            # ---- main-student pass ----
            banks_m = mkbanks("m")
            for b in range(4):
                xtile = loads.tile([128, BLK], bf16, tag="xload")
                nc.sync.dma_start(out=xtile, in_=xsm[:, b * BLK:(b + 1) * BLK])
                sc = scratch.tile([128, BLK], bf16, tag="sc")
                nc.scalar.activation(
                    out=sc, in_=xtile, func=Exp, bias=bias_s, scale=10.0,
                    accum_out=stm[:, b:b + 1])
                colsum(banks_m, sel8t, xtile, b, 4)
            flush(banks_m, o_csm, "m")

            for out_ap, in_ap in deferred:
                nc.sync.dma_start(out=out_ap, in_=in_ap)
            nc.scalar.dma_start(out=o_stat[:, :], in_=stat)

    nc.compile()
    return nc


def _get_program():
    if "nc" not in _CACHE:
        _CACHE["nc"] = _build_program()
    return _CACHE["nc"]


def _selectors():
    sel8 = np.tile(np.eye(8, dtype=np.float32), (16, 1))
    b8 = np.kron(np.eye(16, dtype=np.float32), np.ones((8, 8), np.float32))
    return sel8.astype(ml_dtypes.bfloat16), b8


def _pack_blocks(rows_bf16):
    """[16k, 65536] bf16 -> [128, 8192k]: each 16-row block reshaped to
    [128, 8192] (p = r*8 + s), blocks concatenated along columns."""
    n = rows_bf16.shape[0] // 16
    blks = [np.ascontiguousarray(
        rows_bf16[16 * i:16 * (i + 1)].reshape(128, BLK)) for i in range(n)]
    return np.concatenate(blks, axis=1) if n > 1 else blks[0]


def _unpack_colsum(cs):
    # cs [3, 8, 512*NQ]: [j, s, 512q + c] -> d = s*8192 + (3q+j)*512 + c
    a = np.asarray(cs, dtype=np.float64).reshape(3, 8, NQ, 512)
    vw = np.zeros((8, 16, 512))           # [s, w, c]
    for j in range(3):
        for q in range(NQ):
            w = 3 * q + j
            if w < W:
                vw[:, w, :] = a[j, :, q, :]
    return vw.reshape(D)


def _rowsum(stat_cols, nblk):
    # stat [128, nblk] accum partials, p = r*8 + s -> row = 16*b + r
    a = np.asarray(stat_cols, dtype=np.float64).reshape(16, 8, nblk)
    return a.sum(axis=1).T.reshape(16 * nblk)     # [b*16 + r]


def kernel(student_output, teacher_output, center, epoch):
    from concourse.bass_utils import run_bass_kernel_spmd

    global LAST_RESULTS

    S = np.asarray(student_output, dtype=np.float32).reshape(-1, D)   # [640, D]
    T = np.asarray(teacher_output, dtype=np.float32).reshape(-1, D)   # [128, D]
    cen = np.asarray(center, dtype=np.float32).reshape(1, D)
    ep = int(np.asarray(epoch))
    if ep < 30:
        t_temp = 0.04 + (0.07 - 0.04) * ep / 30
    else:
        t_temp = 0.07

    # host prep: fold center + temperature so the device uses one scale (10)
    tpre = ((T - cen) * np.float32(1.0 / (t_temp * 10.0))).astype(
        ml_dtypes.bfloat16)
    Sb = S.astype(ml_dtypes.bfloat16)

    sel8, b8 = _selectors()
    in_maps = []
    for k in range(NCORES):
        in_maps.append({
            "xt": _pack_blocks(tpre[16 * k:16 * (k + 1)]),
            "xst": _pack_blocks(Sb[16 * k:16 * (k + 1)]),
            "xsm": _pack_blocks(Sb[128 + 64 * k:128 + 64 * (k + 1)]),
            "b8": b8, "sel8": sel8,
        })

    nc = _get_program()
    res = run_bass_kernel_spmd(
        nc, in_maps, core_ids=list(range(NCORES)), trace=TRACE)
    LAST_RESULTS = res

    # host combine, all in float64
    colsum_P = np.zeros(D)
    colsum_sraw = np.zeros(D)
    C = 0.0
    C128 = 0.0
    TR = 0.0
    for k in range(NCORES):
        r = res.results[k]
        colsum_P += _unpack_colsum(r["csp"])
        colsum_sraw += _unpack_colsum(r["cst"])
        colsum_sraw += _unpack_colsum(r["csm"])

        stat = np.asarray(r["stat"], dtype=np.float64)
        z = _rowsum(stat[:, 1:2], 1)          # col1 = both teacher halves
        zs_tr = _rowsum(stat[:, 2:3] + stat[:, 3:4], 1)
        zs_m = _rowsum(stat[:, 4:8], 4)       # main-student exp sums
        tr_acc = _rowsum(stat[:, 8:9] + stat[:, 9:10], 1)

        c_tr = KS + np.log(zs_tr)
        c_m = KS + np.log(zs_m)
        C += c_tr.sum() + c_m.sum()
        C128 += c_tr.sum()
        TR += (10.0 * tr_acc / z).sum()

    colsum_s = 10.0 * colsum_sraw
    s_pl = colsum_P @ colsum_s
    total = s_pl - 128.0 * C - TR + C128
    loss = -total / (128.0 * 639.0)
    return np.array(loss, dtype=np.float32)
